# revision 1
# baseline (speedup 1.0000x reference)
"""Trainium2 Bass kernel for nn_Encoder (3-layer 'bidirectional' LSTM + conv head).

Strategy: data-parallel over batch (8 cores x B_local=8). Per layer, the
recurrence runs as a scan with both directions packed side-by-side in the
free dimension; input projections for layers 1-2 are precomputed as large
matmuls (Gx) and injected into the per-step PSUM accumulation via a tiny
identity matmul. All matmuls run as float32r (full-rate fp32-relaxed).

Four device launches per run: L0 scan -> mid-layer (bulk Gx + scan) x2 ->
conv + argmax.
"""

import numpy as np

import concourse.bass as bass
import concourse.tile as tile
from concourse import bacc, mybir
from concourse.bass_utils import run_bass_kernel_spmd

F32 = mybir.dt.float32
F32R = mybir.dt.float32r
F16 = mybir.dt.float16
I32 = mybir.dt.int32
import os
SCAN_DT = F16 if os.environ.get("KERNEL_SCAN_DT", "f16") == "f16" else F32R
SCAN_NP = np.float16 if SCAN_DT == F16 else np.float32
SCAN_P32 = SCAN_DT == F16 and os.environ.get("KERNEL_P32", "1") == "1"
AF = mybir.ActivationFunctionType

NCORES = 8
B = 64
BL = B // NCORES          # batch per core
D_IN = 64
H = 256
G = 4 * H                 # 1024 gate cols per direction
T = 1024
NCLS = 81
CH = 16                   # scan steps per chunk-buffer half (body = 2*CH)

# gate reorder: pytorch [i f g o] -> [i f o g] so sigmoid cols are contiguous
PERM = np.concatenate([np.arange(0, H), np.arange(H, 2 * H),
                       np.arange(3 * H, 4 * H), np.arange(2 * H, 3 * H)])

_prog_cache = {}


def _scan_layer(nc, tc, *, l0, T, xt=None, wx0=None, gx_dram=None, whh_sb=None,
                zeros=None, ident8=None, ifout_dram=None):
    """Emit the recurrent scan for one layer (both directions).

    PSUM gate layout per step: [8, 2048] = [d0: i f o g | d1: i f o g].
    h-state is kept transposed in chunk buffers cb[4] = (d,k) tiles
    [128, 2*CH*8]; slots alternate X half (0..CH-1) and Y half (CH..2CH-1).
    """
    import contextlib
    ctx = contextlib.ExitStack()
    with ctx:
        psg = ctx.enter_context(tc.tile_pool(name="psg", bufs=1, space="PSUM"))
        pst = ctx.enter_context(tc.tile_pool(name="pst", bufs=2, space="PSUM"))
        sbp = ctx.enter_context(tc.tile_pool(name="sbp", bufs=2))
        gxp = ctx.enter_context(tc.tile_pool(name="gxp", bufs=8))
        cbp = ctx.enter_context(tc.tile_pool(name="cbp", bufs=1))
        stp = ctx.enter_context(tc.tile_pool(name="stp", bufs=1))

        # persistent state
        c = stp.tile([BL, 2 * H], F32)          # cell state [d0 256 | d1 256]
        nc.gpsimd.memset(c[:], 0.0)
        cb = [cbp.tile([128, 2 * CH * BL], SCAN_DT, tag=f"cb{j}", name=f"cb{j}") for j in range(4)]

        nbody = T // (2 * CH)
        for body in range(nbody):
            for half in range(2):
                for j in range(CH):
                    t = body * 2 * CH + half * CH + j
                    slot = half * CH + j
                    pslot = (slot - 1) % (2 * CH)

                    if gx_dram is not None:
                        gxs = gxp.tile([BL, 2 * G], SCAN_DT, tag="gxs")
                        nc.sync.dma_start(gxs[:], gx_dram[t * BL:(t + 1) * BL, :])

                    p = psg.tile([BL, 2 * G], F32, tag="gates")
                    for s in range(4):
                        d, hf = divmod(s, 2)
                        ps = p[:, s * 512:(s + 1) * 512]
                        for k in range(2):
                            if t == 0:
                                lhsT = zeros[:, 0:BL]
                            else:
                                lhsT = cb[2 * d + k][:, pslot * BL:(pslot + 1) * BL]
                            nc.tensor.matmul(ps, lhsT,
                                             whh_sb[k][:, s * 512:(s + 1) * 512],
                                             start=(k == 0), stop=False)
                        if l0:
                            nc.tensor.matmul(ps, xt[:, t * BL:(t + 1) * BL],
                                             wx0[:, s * 512:(s + 1) * 512],
                                             start=False, stop=True)
                        else:
                            nc.tensor.matmul(ps, ident8[0:BL, 0:BL],
                                             gxs[:, s * 512:(s + 1) * 512],
                                             start=False, stop=True)

                    # elementwise: sigmoid(i,f,o), tanh(g)
                    p3 = p[:].rearrange("p (d g) -> p d g", d=2)
                    sig = sbp.tile([BL, 2, 3 * H], F32, tag="sig")
                    nc.scalar.activation(sig[:], p3[:, :, 0:3 * H], AF.Sigmoid)
                    tg = sbp.tile([BL, 2, H], F32, tag="tg")
                    nc.scalar.activation(tg[:], p3[:, :, 3 * H:4 * H], AF.Tanh)

                    c3 = c[:].rearrange("p (d h) -> p d h", d=2)
                    u = sbp.tile([BL, 2, H], F32, tag="u")
                    nc.vector.tensor_mul(u[:], sig[:, :, 0:H], tg[:])
                    nc.vector.tensor_mul(c3, sig[:, :, H:2 * H], c3)
                    nc.vector.tensor_add(c3, c3, u[:])
                    tcy = sbp.tile([BL, 2, H], F32, tag="tcy")
                    nc.scalar.activation(tcy[:], c3, AF.Tanh)
                    hh16 = sbp.tile([BL, 2 * H], SCAN_DT, tag="hh16")
                    h3 = hh16[:].rearrange("p (d h) -> p d h", d=2)
                    nc.vector.tensor_mul(h3, sig[:, :, 2 * H:3 * H], tcy[:])

                    # transpose h -> cb[(d,k)][:, slot]
                    for jj in range(4):
                        d, k = divmod(jj, 2)
                        tp = pst.tile([128, BL], SCAN_DT, tag="tp")
                        hslice = hh16[0:BL, jj * 128:(jj + 1) * 128]
                        nc.tensor.transpose(tp[:], hslice, ident8[0:BL, 0:BL])
                        nc.vector.tensor_copy(cb[2 * d + k][:, slot * BL:(slot + 1) * BL],
                                              tp[:])
                if ifout_dram is not None:
                    c0 = (body * 2 * CH + half * CH) * BL
                    for jj in range(4):
                        nc.sync.dma_start(
                            ifout_dram[jj][:, c0:c0 + CH * BL],
                            cb[jj][:, half * CH * BL:(half + 1) * CH * BL])


def _scan_layer_p32(nc, tc, *, l0, T, xt=None, wx0=None, gx_dram=None, whh_sb=None,
                    zeros=None, ident8=None, ifout_dram=None, body_cb=None):
    """Pad32 scan: directions on partitions 0-7 / 32-39 of a [64, 1024] PSUM,
    col-tiled so the two directions' matmuls run concurrently. fp16 only.
    Junk rows 8-31/40-63 carry garbage (bounded); real rows are extracted on
    transpose-copy. cb[k] cols = slot*16 + d*8 + b, with a 32-col junk tail
    absorbing lhsT access-pattern spill."""
    import contextlib
    ctx = contextlib.ExitStack()
    with ctx:
        psg = ctx.enter_context(tc.tile_pool(name="psg", bufs=2, space="PSUM"))
        pst = ctx.enter_context(tc.tile_pool(name="pst", bufs=2, space="PSUM"))
        sbp = ctx.enter_context(tc.tile_pool(name="sbp", bufs=2))
        gxp = ctx.enter_context(tc.tile_pool(name="gxp", bufs=8))
        cbp = ctx.enter_context(tc.tile_pool(name="cbp", bufs=1))
        stp = ctx.enter_context(tc.tile_pool(name="stp", bufs=1))

        c = stp.tile([64, H], F32)
        nc.gpsimd.memset(c[:], 0.0)
        cbw = 2 * CH * 16
        cb = [cbp.tile([128, cbw + 32], SCAN_DT, tag=f"cb{k}", name=f"cb{k}")
              for k in range(2)]

        def spill32(tile_ap, col0):
            # 32 cols starting at col0, stride 1 (24 junk cols spill right)
            return tile_ap[:, col0:col0 + 32]

        nbody = T // (2 * CH)
        for body in range(nbody):
            if body_cb is not None:
                body_cb(body)
            for half2 in range(2):
                for j in range(CH):
                    t = body * 2 * CH + half2 * CH + j
                    slot = half2 * CH + j
                    pslot = (slot - 1) % (2 * CH)

                    if gx_dram is not None:
                        gxs = gxp.tile([BL, 2 * G], SCAN_DT, tag="gxs")
                        nc.sync.dma_start(gxs[:], gx_dram[t * BL:(t + 1) * BL, :])

                    p = psg.tile([64, G], F32, tag="gates")
                    # input contribution first: independent of h, so it can
                    # overlap the previous step's elementwise tail
                    for hf in range(2):
                        for d in range(2):
                            out = p[d * 32:d * 32 + 32, hf * 512:(hf + 1) * 512]
                            if l0:
                                nc.tensor.matmul(out, xt[:, t * BL:t * BL + 32],
                                                 wx0[:, d * G + hf * 512:d * G + (hf + 1) * 512],
                                                 start=True, stop=False,
                                                 tile_position=(0, d * 32))
                            else:
                                nc.tensor.matmul(out, ident8[0:8, 0:32],
                                                 gxs[:, d * G + hf * 512:d * G + (hf + 1) * 512],
                                                 start=True, stop=False,
                                                 tile_position=(0, d * 32))
                    for hf in range(2):
                        for d in range(2):
                            out = p[d * 32:d * 32 + 32, hf * 512:(hf + 1) * 512]
                            for k in range(2):
                                if t == 0:
                                    lhsT = zeros[:, 0:32]
                                else:
                                    lhsT = spill32(cb[k], pslot * 16 + d * 8)
                                nc.tensor.matmul(out, lhsT,
                                                 whh_sb[k][:, d * G + hf * 512:d * G + (hf + 1) * 512],
                                                 start=False, stop=(k == 1),
                                                 tile_position=(0, d * 32))

                    sig = sbp.tile([64, 3 * H], SCAN_DT, tag="sig")
                    nc.scalar.activation(sig[:, 0:2 * H], p[0:64, 0:2 * H], AF.Sigmoid)
                    tg = sbp.tile([64, H], SCAN_DT, tag="tg")
                    nc.scalar.activation(tg[:], p[0:64, 3 * H:4 * H], AF.Tanh)
                    nc.scalar.activation(sig[:, 2 * H:3 * H], p[0:64, 2 * H:3 * H],
                                         AF.Sigmoid)
                    u = sbp.tile([64, H], SCAN_DT, tag="u")
                    nc.vector.tensor_mul(u[:], sig[:, 0:H], tg[:])
                    nc.vector.tensor_mul(c[:], sig[:, H:2 * H], c[:])
                    nc.vector.tensor_add(c[:], c[:], u[:])
                    tcy = sbp.tile([64, H], SCAN_DT, tag="tcy")
                    nc.scalar.activation(tcy[:], c[:], AF.Tanh)
                    hh16 = sbp.tile([64, H], SCAN_DT, tag="hh16")
                    nc.vector.tensor_mul(hh16[:], sig[:, 2 * H:3 * H], tcy[:])

                    for k in range(2):
                        tp = pst.tile([128, 64], SCAN_DT, tag="tp")
                        nc.tensor.transpose(tp[:], hh16[0:64, k * 128:(k + 1) * 128],
                                            ident8[:])
                        # extract real cols {0-7, 32-39} -> cb[k] slot cols
                        srcv = tp[:].rearrange("p (d q) -> p d q", d=2)[:, :, 0:8]
                        dstv = cb[k][:, slot * 16:(slot + 1) * 16].rearrange(
                            "p (d b) -> p d b", d=2)
                        nc.vector.tensor_copy(dstv, srcv)
                if ifout_dram is not None:
                    c0 = (body * 2 * CH + half2 * CH) * BL
                    for kk in range(4):
                        d, k = divmod(kk, 2)
                        blk = cb[k][:, half2 * CH * 16:(half2 + 1) * CH * 16]
                        src3 = blk.rearrange("p (s b) -> p s b", b=16)[:, :, d * 8:d * 8 + 8]
                        nc.sync.dma_start(
                            ifout_dram[kk][:, c0:c0 + CH * BL].rearrange(
                                "p (s b) -> p s b", b=8),
                            src3)


def _build_l0(T):
    nc = bacc.Bacc("TRN2", target_bir_lowering=False, debug=False,
                   num_devices=NCORES)
    xt_d = nc.dram_tensor("xt", [D_IN + 1, T * BL + 32], SCAN_DT, kind="ExternalInput").ap()
    wx0_d = nc.dram_tensor("wx0", [D_IN + 1, 2 * G], SCAN_DT, kind="ExternalInput").ap()
    whh_d = nc.dram_tensor("whh", [2, 128, 2 * G], SCAN_DT, kind="ExternalInput").ap()
    z_d = nc.dram_tensor("zeros", [128, 32], SCAN_DT, kind="ExternalInput").ap()
    e8_d = nc.dram_tensor("ident8", [64, 64], SCAN_DT, kind="ExternalInput").ap()
    ifo_d = [nc.dram_tensor(f"ifout{j}", [128, T * BL], SCAN_DT,
                            kind="ExternalOutput").ap() for j in range(4)]

    with tile.TileContext(nc) as tc:
        with tc.tile_pool(name="w", bufs=1) as wp:
            xt = wp.tile([D_IN + 1, T * BL + 32], SCAN_DT)
            nc.sync.dma_start(xt[:], xt_d[:])
            wx0 = wp.tile([D_IN + 1, 2 * G], SCAN_DT)
            nc.sync.dma_start(wx0[:], wx0_d[:])
            whh_sb = [wp.tile([128, 2 * G], SCAN_DT, tag=f"whh{k}", name=f"whh{k}") for k in range(2)]
            for k in range(2):
                nc.sync.dma_start(whh_sb[k][:], whh_d[k])
            zeros = wp.tile([128, 32], SCAN_DT)
            nc.sync.dma_start(zeros[:], z_d[:])
            ident8 = wp.tile([64, 64], SCAN_DT)
            nc.sync.dma_start(ident8[:], e8_d[:])
            fn = _scan_layer_p32 if SCAN_P32 else _scan_layer
            fn(nc, tc, l0=True, T=T, xt=xt, wx0=wx0, whh_sb=whh_sb,
               zeros=zeros, ident8=ident8, ifout_dram=ifo_d)
    nc.compile()
    return nc


def _build_mid(T):
    """Bulk Gx from the input interface, then scan; emits new interface."""
    nc = bacc.Bacc("TRN2", target_bir_lowering=False, debug=False,
                   num_devices=NCORES)
    ifin_d = [nc.dram_tensor(f"ifin{j}", [128, T * BL], SCAN_DT,
                             kind="ExternalInput").ap() for j in range(4)]
    wx_d = nc.dram_tensor("wx", [4, 128, 2 * G], SCAN_DT, kind="ExternalInput").ap()
    bias_d = nc.dram_tensor("bias", [1, 2 * G], SCAN_DT, kind="ExternalInput").ap()
    ones_d = nc.dram_tensor("ones", [1, 128], SCAN_DT, kind="ExternalInput").ap()
    whh_d = nc.dram_tensor("whh", [2, 128, 2 * G], SCAN_DT, kind="ExternalInput").ap()
    z_d = nc.dram_tensor("zeros", [128, 32], SCAN_DT, kind="ExternalInput").ap()
    e8_d = nc.dram_tensor("ident8", [64, 64], SCAN_DT, kind="ExternalInput").ap()
    ifo_d = [nc.dram_tensor(f"ifout{j}", [128, T * BL], SCAN_DT,
                            kind="ExternalOutput").ap() for j in range(4)]


    with tile.TileContext(nc) as tc:
        with tc.tile_pool(name="w", bufs=1) as wp, \
             tc.tile_pool(name="gxdram", bufs=1, space="DRAM") as gdp:
            gx_d = gdp.tile([T * BL, 2 * G], SCAN_DT)
            whh_sb = [wp.tile([128, 2 * G], SCAN_DT, tag=f"whh{k}", name=f"whh{k}") for k in range(2)]
            for k in range(2):
                nc.sync.dma_start(whh_sb[k][:], whh_d[k])
            zeros = wp.tile([128, 32], SCAN_DT)
            nc.sync.dma_start(zeros[:], z_d[:])
            ident8 = wp.tile([64, 64], SCAN_DT)
            nc.sync.dma_start(ident8[:], e8_d[:])

            # ---- bulk Gx, interleaved with the scan (independent work) ----
            import contextlib
            with contextlib.ExitStack() as bctx:
                bwp = bctx.enter_context(tc.tile_pool(name="bw", bufs=1))
                blp = bctx.enter_context(tc.tile_pool(name="bl", bufs=2))
                bpp = bctx.enter_context(tc.tile_pool(name="bp", bufs=2, space="PSUM"))
                bop = bctx.enter_context(tc.tile_pool(name="bo", bufs=2))
                wx_sb = [bwp.tile([128, 2 * G], SCAN_DT, tag=f"wx{k}", name=f"wx{k}") for k in range(4)]
                for k in range(4):
                    nc.sync.dma_start(wx_sb[k][:], wx_d[k])
                bias = bwp.tile([1, 2 * G], SCAN_DT)
                nc.sync.dma_start(bias[:], bias_d[:])
                ones = bwp.tile([1, 128], SCAN_DT)
                nc.sync.dma_start(ones[:], ones_d[:])
                nmt = T * BL // 128

                def bulk_mtile(mt):
                    lhs = [blp.tile([128, 128], SCAN_DT, tag=f"lhs{k}", name=f"lhs{k}") for k in range(4)]
                    for k in range(4):
                        nc.sync.dma_start(lhs[k][:], ifin_d[k][:, mt * 128:(mt + 1) * 128])
                    gout = bop.tile([128, 2 * G], SCAN_DT, tag="gout")
                    for s in range(4):
                        pp = bpp.tile([128, 512], F32, tag="bp")
                        for k in range(4):
                            nc.tensor.matmul(pp[:], lhs[k][:],
                                             wx_sb[k][:, s * 512:(s + 1) * 512],
                                             start=(k == 0), stop=False)
                        nc.tensor.matmul(pp[:], ones[:],
                                         bias[:, s * 512:(s + 1) * 512],
                                         start=False, stop=True)
                        if s % 2 == 0:
                            nc.scalar.activation(gout[:, s * 512:(s + 1) * 512],
                                                 pp[:], AF.Copy)
                        else:
                            nc.vector.tensor_copy(gout[:, s * 512:(s + 1) * 512], pp[:])
                    nc.sync.dma_start(gx_d[mt * 128:(mt + 1) * 128, :], gout[:])

                LEAD = 6
                if SCAN_P32:
                    for mt in range(min(LEAD, nmt)):
                        bulk_mtile(mt)

                    def body_cb(body):
                        for mt in (2 * body + LEAD, 2 * body + LEAD + 1):
                            if mt < nmt:
                                bulk_mtile(mt)
                    _scan_layer_p32(nc, tc, l0=False, T=T, gx_dram=gx_d,
                                    whh_sb=whh_sb, zeros=zeros, ident8=ident8,
                                    ifout_dram=ifo_d, body_cb=body_cb)
                else:
                    for mt in range(nmt):
                        bulk_mtile(mt)
                    _scan_layer(nc, tc, l0=False, T=T, gx_dram=gx_d, whh_sb=whh_sb,
                                zeros=zeros, ident8=ident8, ifout_dram=ifo_d)
    nc.compile()
    return nc


def _build_conv(T):
    """conv head + argmax from the layer-2 interface (d1 tiles)."""
    nc = bacc.Bacc("TRN2", target_bir_lowering=False, debug=False,
                   num_devices=NCORES)
    hs_d = [nc.dram_tensor(f"hseq{j}", [128, T * BL], SCAN_DT,
                           kind="ExternalInput").ap() for j in range(2)]
    cw_d = nc.dram_tensor("convwt", [2, 128, NCLS], SCAN_DT, kind="ExternalInput").ap()
    cb_d = nc.dram_tensor("convb", [NCLS, 1], F32, kind="ExternalInput").ap()
    io_d = nc.dram_tensor("iota", [128, NCLS], F32, kind="ExternalInput").ap()
    e81_d = nc.dram_tensor("ident81", [NCLS, NCLS], F32, kind="ExternalInput").ap()
    out_d = nc.dram_tensor("idx", [128, T * BL // 128], I32, kind="ExternalOutput").ap()

    with tile.TileContext(nc) as tc:
        import contextlib
        with contextlib.ExitStack() as ctx:
            wp = ctx.enter_context(tc.tile_pool(name="w", bufs=1))
            rp = ctx.enter_context(tc.tile_pool(name="r", bufs=2))
            pp_ = ctx.enter_context(tc.tile_pool(name="pp", bufs=2, space="PSUM"))
            tpp = ctx.enter_context(tc.tile_pool(name="tq", bufs=4, space="PSUM"))
            op = ctx.enter_context(tc.tile_pool(name="o", bufs=1))
            cw = [wp.tile([128, NCLS], SCAN_DT, tag=f"cw{k}", name=f"cw{k}") for k in range(2)]
            for k in range(2):
                nc.sync.dma_start(cw[k][:], cw_d[k])
            cbias = wp.tile([NCLS, 1], F32)
            nc.sync.dma_start(cbias[:], cb_d[:])
            iota = wp.tile([128, NCLS], F32)
            nc.sync.dma_start(iota[:], io_d[:])
            e81 = wp.tile([NCLS, NCLS], F32)
            nc.sync.dma_start(e81[:], e81_d[:])
            outsb = op.tile([128, T * BL // 128], F32)
            outi = op.tile([128, T * BL // 128], I32)

            nnt = T * BL // 512
            for nt in range(nnt):
                hseq = [rp.tile([128, 512], SCAN_DT, tag=f"hs{k}", name=f"hs{k}") for k in range(2)]
                for k in range(2):
                    nc.sync.dma_start(hseq[k][:], hs_d[k][:, nt * 512:(nt + 1) * 512])
                lp = pp_.tile([NCLS, 512], F32, tag="lg")
                for k in range(2):
                    nc.tensor.matmul(lp[:], cw[k][:], hseq[k][:],
                                     start=(k == 0), stop=(k == 1))
                lg = rp.tile([NCLS, 512], F32, tag="lgs")
                nc.scalar.activation(lg[:], lp[:], AF.Identity, bias=cbias[:])
                for q in range(4):
                    ch = nt * 4 + q
                    tp = tpp.tile([128, NCLS], F32, tag="tr")
                    nc.tensor.transpose(tp[:], lg[0:NCLS, q * 128:(q + 1) * 128], e81[:])
                    lt = rp.tile([128, NCLS], F32, tag="lt")
                    nc.vector.tensor_copy(lt[:], tp[:])
                    mx = rp.tile([128, 1], F32, tag="mx")
                    nc.vector.reduce_max(mx[:], lt[:], axis=mybir.AxisListType.X)
                    msk = rp.tile([128, NCLS], F32, tag="msk")
                    nc.vector.tensor_scalar(msk[:], lt[:], mx[:], None,
                                            mybir.AluOpType.is_equal)
                    nc.vector.tensor_mul(msk[:], msk[:], iota[:])
                    nc.vector.reduce_max(outsb[:, ch:ch + 1], msk[:],
                                         axis=mybir.AxisListType.X)
            nc.vector.tensor_copy(outi[:], outsb[:])
            # raw layout [p=tt*8+b, ch]; host unscrambles to [b, t=ch*16+tt]
            nc.sync.dma_start(out_d, outi[:])
    nc.compile()
    return nc


def _get_progs(T):
    if T not in _prog_cache:
        _prog_cache[T] = (_build_l0(T), _build_mid(T), _build_conv(T))
    return _prog_cache[T]


def _prep_weights(Wih0, Whh0, b0, Wih12, Whh12, b12, conv_w, conv_b):
    """Host-side packing. All gate rows permuted to [i f o g]."""
    f = np.float32
    wx0 = np.zeros((D_IN + 1, 2 * G), f)
    whh0 = np.zeros((2, 128, 2 * G), f)
    for d in range(2):
        wx0[0:D_IN, d * G:(d + 1) * G] = Wih0[d][PERM].T
        wx0[D_IN, d * G:(d + 1) * G] = b0[d][PERM]
        wt = Whh0[d][PERM].T  # [256, 1024]
        for k in range(2):
            whh0[k, :, d * G:(d + 1) * G] = wt[k * 128:(k + 1) * 128]
    wx12 = np.zeros((2, 4, 128, 2 * G), f)
    bias12 = np.zeros((2, 1, 2 * G), f)
    whh12 = np.zeros((2, 2, 128, 2 * G), f)
    for li in range(2):
        for d in range(2):
            wt = Wih12[li, d][PERM].T  # [512, 1024]
            for k in range(4):
                wx12[li, k, :, d * G:(d + 1) * G] = wt[k * 128:(k + 1) * 128]
            bias12[li, 0, d * G:(d + 1) * G] = b12[li, d][PERM]
            ht = Whh12[li, d][PERM].T  # [256, 1024]
            for k in range(2):
                whh12[li, k, :, d * G:(d + 1) * G] = ht[k * 128:(k + 1) * 128]
    cwt = np.zeros((2, 128, NCLS), f)
    for k in range(2):
        cwt[k] = conv_w.T[k * 128:(k + 1) * 128]
    return wx0, whh0, wx12, bias12, whh12, cwt


def _launch(nc, in_maps, cores, trace):
    r = run_bass_kernel_spmd(nc, in_maps, cores, trace=trace)
    ns = r.exec_time_ns if trace else None
    return r.results, (ns or 0)


def _run(x, Wih0, Whh0, b0, Wih12, Whh12, b12, conv_w, conv_b, trace=False):
    x = np.asarray(x, np.float32)
    args = [np.asarray(a, np.float32) for a in
            (Wih0, Whh0, b0, Wih12, Whh12, b12, conv_w, conv_b)]
    wx0, whh0, wx12, bias12, whh12, cwt = _prep_weights(*args)
    conv_b = args[7]

    T_ = x.shape[2]
    progs = _get_progs(T_)
    nc_l0, nc_mid, nc_conv = progs
    cores = list(range(NCORES))

    zeros = np.zeros((128, 32), SCAN_NP)
    ident8 = np.eye(64, dtype=SCAN_NP)
    ones = np.ones((1, 128), SCAN_NP)
    iota = np.tile(np.arange(NCLS, dtype=np.float32), (128, 1))
    ident81 = np.eye(NCLS, dtype=np.float32)

    # xt per core: [65, T*8], col = t*8 + b, row 64 = ones
    xt_all = []
    for ci in range(NCORES):
        xs = x[ci * BL:(ci + 1) * BL]          # [8, 64, T]
        xt = np.zeros((D_IN + 1, T_ * BL + 32), SCAN_NP)
        xt[0:D_IN, 0:T_ * BL] = xs.transpose(1, 2, 0).reshape(D_IN, T_ * BL)
        xt[D_IN, 0:T_ * BL] = 1.0
        xt_all.append(xt)

    total_ns = 0
    r0, ns = _launch(nc_l0, [
        {"xt": xt_all[ci], "wx0": wx0.astype(SCAN_NP), "whh": whh0.astype(SCAN_NP), "zeros": zeros,
         "ident8": ident8} for ci in cores], cores, trace)
    total_ns += ns

    cur = r0
    for li in range(2):
        rm, ns = _launch(nc_mid, [
            {**{f"ifin{j}": cur[ci][f"ifout{j}"] for j in range(4)},
             "wx": wx12[li].astype(SCAN_NP), "bias": bias12[li].astype(SCAN_NP), "ones": ones,
             "whh": whh12[li].astype(SCAN_NP), "zeros": zeros, "ident8": ident8}
            for ci in cores], cores, trace)
        cur = rm
        total_ns += ns

    rc, ns = _launch(nc_conv, [
        {"hseq0": cur[ci]["ifout2"], "hseq1": cur[ci]["ifout3"],
         "convwt": cwt.astype(SCAN_NP), "convb": conv_b.reshape(NCLS, 1).astype(np.float32),
         "iota": iota, "ident81": ident81} for ci in cores], cores, trace)
    total_ns += ns

    outs = []
    for ci in cores:
        raw = rc[ci]["idx"]                      # [128, T/16]: [tt*8+b, ch]
        o = raw.reshape(16, BL, T_ // 16).transpose(1, 2, 0).reshape(BL, T_)
        outs.append(o)
    return np.concatenate(outs, axis=0).astype(np.int32), total_ns


def kernel(**inputs):
    out, _ = _run(**inputs)
    return out


def profiled_run(**inputs):
    _, ns = _run(**inputs, trace=True)
    return ns



# revision 2
# speedup vs baseline: 1371.4262x; 1371.4262x over previous
"""Trainium2 Bass kernel for nn_Encoder (3-layer 'bidirectional' LSTM + conv head).

Strategy: data-parallel over batch (8 cores x B_local=8). Per layer, the
recurrence runs as a scan with both directions packed side-by-side in the
free dimension; input projections for layers 1-2 are precomputed as large
matmuls (Gx) and injected into the per-step PSUM accumulation via a tiny
identity matmul. All matmuls run as float32r (full-rate fp32-relaxed).

Four device launches per run: L0 scan -> mid-layer (bulk Gx + scan) x2 ->
conv + argmax.
"""

import numpy as np

import concourse.bass as bass
import concourse.tile as tile
from concourse import bacc, mybir
from concourse.bass_utils import run_bass_kernel_spmd

F32 = mybir.dt.float32
F32R = mybir.dt.float32r
F16 = mybir.dt.float16
I32 = mybir.dt.int32
import os
SCAN_DT = F16 if os.environ.get("KERNEL_SCAN_DT", "f16") == "f16" else F32R
SCAN_NP = np.float16 if SCAN_DT == F16 else np.float32
SCAN_P32 = SCAN_DT == F16 and os.environ.get("KERNEL_P32", "1") == "1"
AF = mybir.ActivationFunctionType

NCORES = 8
B = 64
BL = B // NCORES          # batch per core
D_IN = 64
H = 256
G = 4 * H                 # 1024 gate cols per direction
T = 1024
NCLS = 81
CH = 16                   # scan steps per chunk-buffer half (body = 2*CH)

# gate reorder: pytorch [i f g o] -> [i f o g] so sigmoid cols are contiguous
PERM = np.concatenate([np.arange(0, H), np.arange(H, 2 * H),
                       np.arange(3 * H, 4 * H), np.arange(2 * H, 3 * H)])

_prog_cache = {}


def _scan_layer(nc, tc, *, l0, T, xt=None, wx0=None, gx_dram=None, whh_sb=None,
                zeros=None, ident8=None, ifout_dram=None):
    """Emit the recurrent scan for one layer (both directions).

    PSUM gate layout per step: [8, 2048] = [d0: i f o g | d1: i f o g].
    h-state is kept transposed in chunk buffers cb[4] = (d,k) tiles
    [128, 2*CH*8]; slots alternate X half (0..CH-1) and Y half (CH..2CH-1).
    """
    import contextlib
    ctx = contextlib.ExitStack()
    with ctx:
        psg = ctx.enter_context(tc.tile_pool(name="psg", bufs=1, space="PSUM"))
        pst = ctx.enter_context(tc.tile_pool(name="pst", bufs=2, space="PSUM"))
        sbp = ctx.enter_context(tc.tile_pool(name="sbp", bufs=2))
        gxp = ctx.enter_context(tc.tile_pool(name="gxp", bufs=8))
        cbp = ctx.enter_context(tc.tile_pool(name="cbp", bufs=1))
        stp = ctx.enter_context(tc.tile_pool(name="stp", bufs=1))

        # persistent state
        c = stp.tile([BL, 2 * H], F32)          # cell state [d0 256 | d1 256]
        nc.gpsimd.memset(c[:], 0.0)
        cb = [cbp.tile([128, 2 * CH * BL], SCAN_DT, tag=f"cb{j}", name=f"cb{j}") for j in range(4)]

        nbody = T // (2 * CH)
        for body in range(nbody):
            for half in range(2):
                for j in range(CH):
                    t = body * 2 * CH + half * CH + j
                    slot = half * CH + j
                    pslot = (slot - 1) % (2 * CH)

                    if gx_dram is not None:
                        gxs = gxp.tile([BL, 2 * G], SCAN_DT, tag="gxs")
                        nc.sync.dma_start(gxs[:], gx_dram[t * BL:(t + 1) * BL, :])

                    p = psg.tile([BL, 2 * G], F32, tag="gates")
                    for s in range(4):
                        d, hf = divmod(s, 2)
                        ps = p[:, s * 512:(s + 1) * 512]
                        for k in range(2):
                            if t == 0:
                                lhsT = zeros[:, 0:BL]
                            else:
                                lhsT = cb[2 * d + k][:, pslot * BL:(pslot + 1) * BL]
                            nc.tensor.matmul(ps, lhsT,
                                             whh_sb[k][:, s * 512:(s + 1) * 512],
                                             start=(k == 0), stop=False)
                        if l0:
                            nc.tensor.matmul(ps, xt[:, t * BL:(t + 1) * BL],
                                             wx0[:, s * 512:(s + 1) * 512],
                                             start=False, stop=True)
                        else:
                            nc.tensor.matmul(ps, ident8[0:BL, 0:BL],
                                             gxs[:, s * 512:(s + 1) * 512],
                                             start=False, stop=True)

                    # elementwise: sigmoid(i,f,o), tanh(g)
                    p3 = p[:].rearrange("p (d g) -> p d g", d=2)
                    sig = sbp.tile([BL, 2, 3 * H], F32, tag="sig")
                    nc.scalar.activation(sig[:], p3[:, :, 0:3 * H], AF.Sigmoid)
                    tg = sbp.tile([BL, 2, H], F32, tag="tg")
                    nc.scalar.activation(tg[:], p3[:, :, 3 * H:4 * H], AF.Tanh)

                    c3 = c[:].rearrange("p (d h) -> p d h", d=2)
                    u = sbp.tile([BL, 2, H], F32, tag="u")
                    nc.vector.tensor_mul(u[:], sig[:, :, 0:H], tg[:])
                    nc.vector.tensor_mul(c3, sig[:, :, H:2 * H], c3)
                    nc.vector.tensor_add(c3, c3, u[:])
                    tcy = sbp.tile([BL, 2, H], F32, tag="tcy")
                    nc.scalar.activation(tcy[:], c3, AF.Tanh)
                    hh16 = sbp.tile([BL, 2 * H], SCAN_DT, tag="hh16")
                    h3 = hh16[:].rearrange("p (d h) -> p d h", d=2)
                    nc.vector.tensor_mul(h3, sig[:, :, 2 * H:3 * H], tcy[:])

                    # transpose h -> cb[(d,k)][:, slot]
                    for jj in range(4):
                        d, k = divmod(jj, 2)
                        tp = pst.tile([128, BL], SCAN_DT, tag="tp")
                        hslice = hh16[0:BL, jj * 128:(jj + 1) * 128]
                        nc.tensor.transpose(tp[:], hslice, ident8[0:BL, 0:BL])
                        nc.vector.tensor_copy(cb[2 * d + k][:, slot * BL:(slot + 1) * BL],
                                              tp[:])
                if ifout_dram is not None:
                    c0 = (body * 2 * CH + half * CH) * BL
                    for jj in range(4):
                        nc.sync.dma_start(
                            ifout_dram[jj][:, c0:c0 + CH * BL],
                            cb[jj][:, half * CH * BL:(half + 1) * CH * BL])


def _scan_layer_p32(nc, tc, *, l0, T, xt=None, wx0=None, gx_dram=None, whh_sb=None,
                    zeros=None, ident8=None, ifout_dram=None, body_cb=None):
    """Pad32 scan: directions on partitions 0-7 / 32-39 of a [64, 1024] PSUM,
    col-tiled so the two directions' matmuls run concurrently. fp16 only.
    Junk rows 8-31/40-63 carry garbage (bounded); real rows are extracted on
    transpose-copy. cb[k] cols = slot*16 + d*8 + b, with a 32-col junk tail
    absorbing lhsT access-pattern spill."""
    import contextlib
    ctx = contextlib.ExitStack()
    with ctx:
        psg = ctx.enter_context(tc.tile_pool(name="psg", bufs=2, space="PSUM"))
        pst = ctx.enter_context(tc.tile_pool(name="pst", bufs=2, space="PSUM"))
        sbp = ctx.enter_context(tc.tile_pool(name="sbp", bufs=2))
        gxp = ctx.enter_context(tc.tile_pool(name="gxp", bufs=8))
        cbp = ctx.enter_context(tc.tile_pool(name="cbp", bufs=1))
        stp = ctx.enter_context(tc.tile_pool(name="stp", bufs=1))

        c = stp.tile([64, H], F32)
        nc.gpsimd.memset(c[:], 0.0)
        cbw = 2 * CH * 16
        cb = [cbp.tile([128, cbw + 32], SCAN_DT, tag=f"cb{k}", name=f"cb{k}")
              for k in range(2)]

        def spill32(tile_ap, col0):
            # 32 cols starting at col0, stride 1 (24 junk cols spill right)
            return tile_ap[:, col0:col0 + 32]

        nbody = T // (2 * CH)
        for body in range(nbody):
            if body_cb is not None:
                body_cb(body)
            for half2 in range(2):
                for j in range(CH):
                    t = body * 2 * CH + half2 * CH + j
                    slot = half2 * CH + j
                    pslot = (slot - 1) % (2 * CH)

                    if gx_dram is not None:
                        gxs = gxp.tile([BL, 2 * G], SCAN_DT, tag="gxs")
                        nc.sync.dma_start(gxs[:], gx_dram[t * BL:(t + 1) * BL, :])

                    p = psg.tile([64, G], F32, tag="gates")
                    # input contribution first: independent of h, so it can
                    # overlap the previous step's elementwise tail
                    for hf in range(2):
                        for d in range(2):
                            out = p[d * 32:d * 32 + 32, hf * 512:(hf + 1) * 512]
                            if l0:
                                nc.tensor.matmul(out, xt[:, t * BL:t * BL + 32],
                                                 wx0[:, d * G + hf * 512:d * G + (hf + 1) * 512],
                                                 start=True, stop=False,
                                                 tile_position=(0, d * 32))
                            else:
                                nc.tensor.matmul(out, ident8[0:8, 0:32],
                                                 gxs[:, d * G + hf * 512:d * G + (hf + 1) * 512],
                                                 start=True, stop=False,
                                                 tile_position=(0, d * 32))
                    for hf in range(2):
                        for d in range(2):
                            out = p[d * 32:d * 32 + 32, hf * 512:(hf + 1) * 512]
                            for k in range(2):
                                if t == 0:
                                    lhsT = zeros[:, 0:32]
                                else:
                                    lhsT = spill32(cb[k], pslot * 16 + d * 8)
                                nc.tensor.matmul(out, lhsT,
                                                 whh_sb[k][:, d * G + hf * 512:d * G + (hf + 1) * 512],
                                                 start=False, stop=(k == 1),
                                                 tile_position=(0, d * 32))

                    sig = sbp.tile([64, 3 * H], SCAN_DT, tag="sig")
                    nc.scalar.activation(sig[:, 0:2 * H], p[0:64, 0:2 * H], AF.Sigmoid)
                    tg = sbp.tile([64, H], SCAN_DT, tag="tg")
                    nc.scalar.activation(tg[:], p[0:64, 3 * H:4 * H], AF.Tanh)
                    nc.scalar.activation(sig[:, 2 * H:3 * H], p[0:64, 2 * H:3 * H],
                                         AF.Sigmoid)
                    u = sbp.tile([64, H], SCAN_DT, tag="u")
                    nc.vector.tensor_mul(u[:], sig[:, 0:H], tg[:])
                    nc.vector.tensor_mul(c[:], sig[:, H:2 * H], c[:])
                    nc.vector.tensor_add(c[:], c[:], u[:])
                    tcy = sbp.tile([64, H], SCAN_DT, tag="tcy")
                    nc.scalar.activation(tcy[:], c[:], AF.Tanh)
                    hh16 = sbp.tile([64, H], SCAN_DT, tag="hh16")
                    nc.vector.tensor_mul(hh16[:], sig[:, 2 * H:3 * H], tcy[:])

                    for k in range(2):
                        tp = pst.tile([128, 64], SCAN_DT, tag="tp")
                        nc.tensor.transpose(tp[:], hh16[0:64, k * 128:(k + 1) * 128],
                                            ident8[:])
                        # extract real cols {0-7, 32-39} -> cb[k] slot cols
                        srcv = tp[:].rearrange("p (d q) -> p d q", d=2)[:, :, 0:8]
                        dstv = cb[k][:, slot * 16:(slot + 1) * 16].rearrange(
                            "p (d b) -> p d b", d=2)
                        nc.vector.tensor_copy(dstv, srcv)
                if ifout_dram is not None:
                    c0 = (body * 2 * CH + half2 * CH) * BL
                    for kk in range(4):
                        d, k = divmod(kk, 2)
                        blk = cb[k][:, half2 * CH * 16:(half2 + 1) * CH * 16]
                        src3 = blk.rearrange("p (s b) -> p s b", b=16)[:, :, d * 8:d * 8 + 8]
                        nc.sync.dma_start(
                            ifout_dram[kk][:, c0:c0 + CH * BL].rearrange(
                                "p (s b) -> p s b", b=8),
                            src3)


def _build_l0(T):
    nc = bacc.Bacc("TRN2", target_bir_lowering=False, debug=False,
                   num_devices=NCORES)
    xt_d = nc.dram_tensor("xt", [D_IN + 1, T * BL + 32], SCAN_DT, kind="ExternalInput").ap()
    wx0_d = nc.dram_tensor("wx0", [D_IN + 1, 2 * G], SCAN_DT, kind="ExternalInput").ap()
    whh_d = nc.dram_tensor("whh", [2, 128, 2 * G], SCAN_DT, kind="ExternalInput").ap()
    z_d = nc.dram_tensor("zeros", [128, 32], SCAN_DT, kind="ExternalInput").ap()
    e8_d = nc.dram_tensor("ident8", [64, 64], SCAN_DT, kind="ExternalInput").ap()
    ifo_d = [nc.dram_tensor(f"ifout{j}", [128, T * BL], SCAN_DT,
                            kind="ExternalOutput").ap() for j in range(4)]

    with tile.TileContext(nc) as tc:
        with tc.tile_pool(name="w", bufs=1) as wp:
            xt = wp.tile([D_IN + 1, T * BL + 32], SCAN_DT)
            nc.sync.dma_start(xt[:], xt_d[:])
            wx0 = wp.tile([D_IN + 1, 2 * G], SCAN_DT)
            nc.sync.dma_start(wx0[:], wx0_d[:])
            whh_sb = [wp.tile([128, 2 * G], SCAN_DT, tag=f"whh{k}", name=f"whh{k}") for k in range(2)]
            for k in range(2):
                nc.sync.dma_start(whh_sb[k][:], whh_d[k])
            zeros = wp.tile([128, 32], SCAN_DT)
            nc.sync.dma_start(zeros[:], z_d[:])
            ident8 = wp.tile([64, 64], SCAN_DT)
            nc.sync.dma_start(ident8[:], e8_d[:])
            fn = _scan_layer_p32 if SCAN_P32 else _scan_layer
            fn(nc, tc, l0=True, T=T, xt=xt, wx0=wx0, whh_sb=whh_sb,
               zeros=zeros, ident8=ident8, ifout_dram=ifo_d)
    nc.compile()
    return nc


def _build_mid(T):
    """Bulk Gx from the input interface, then scan; emits new interface."""
    nc = bacc.Bacc("TRN2", target_bir_lowering=False, debug=False,
                   num_devices=NCORES)
    ifin_d = [nc.dram_tensor(f"ifin{j}", [128, T * BL], SCAN_DT,
                             kind="ExternalInput").ap() for j in range(4)]
    wx_d = nc.dram_tensor("wx", [4, 128, 2 * G], SCAN_DT, kind="ExternalInput").ap()
    bias_d = nc.dram_tensor("bias", [1, 2 * G], SCAN_DT, kind="ExternalInput").ap()
    ones_d = nc.dram_tensor("ones", [1, 128], SCAN_DT, kind="ExternalInput").ap()
    whh_d = nc.dram_tensor("whh", [2, 128, 2 * G], SCAN_DT, kind="ExternalInput").ap()
    z_d = nc.dram_tensor("zeros", [128, 32], SCAN_DT, kind="ExternalInput").ap()
    e8_d = nc.dram_tensor("ident8", [64, 64], SCAN_DT, kind="ExternalInput").ap()
    ifo_d = [nc.dram_tensor(f"ifout{j}", [128, T * BL], SCAN_DT,
                            kind="ExternalOutput").ap() for j in range(4)]


    with tile.TileContext(nc) as tc:
        with tc.tile_pool(name="w", bufs=1) as wp, \
             tc.tile_pool(name="gxdram", bufs=1, space="DRAM") as gdp:
            gx_d = gdp.tile([T * BL, 2 * G], SCAN_DT)
            whh_sb = [wp.tile([128, 2 * G], SCAN_DT, tag=f"whh{k}", name=f"whh{k}") for k in range(2)]
            for k in range(2):
                nc.sync.dma_start(whh_sb[k][:], whh_d[k])
            zeros = wp.tile([128, 32], SCAN_DT)
            nc.sync.dma_start(zeros[:], z_d[:])
            ident8 = wp.tile([64, 64], SCAN_DT)
            nc.sync.dma_start(ident8[:], e8_d[:])

            # ---- bulk Gx, interleaved with the scan (independent work) ----
            import contextlib
            with contextlib.ExitStack() as bctx:
                bwp = bctx.enter_context(tc.tile_pool(name="bw", bufs=1))
                blp = bctx.enter_context(tc.tile_pool(name="bl", bufs=2))
                bpp = bctx.enter_context(tc.tile_pool(name="bp", bufs=2, space="PSUM"))
                bop = bctx.enter_context(tc.tile_pool(name="bo", bufs=2))
                wx_sb = [bwp.tile([128, 2 * G], SCAN_DT, tag=f"wx{k}", name=f"wx{k}") for k in range(4)]
                for k in range(4):
                    nc.sync.dma_start(wx_sb[k][:], wx_d[k])
                bias = bwp.tile([1, 2 * G], SCAN_DT)
                nc.sync.dma_start(bias[:], bias_d[:])
                ones = bwp.tile([1, 128], SCAN_DT)
                nc.sync.dma_start(ones[:], ones_d[:])
                nmt = T * BL // 128

                def bulk_mtile(mt):
                    lhs = [blp.tile([128, 128], SCAN_DT, tag=f"lhs{k}", name=f"lhs{k}") for k in range(4)]
                    for k in range(4):
                        nc.sync.dma_start(lhs[k][:], ifin_d[k][:, mt * 128:(mt + 1) * 128])
                    gout = bop.tile([128, 2 * G], SCAN_DT, tag="gout")
                    for s in range(4):
                        pp = bpp.tile([128, 512], F32, tag="bp")
                        for k in range(4):
                            nc.tensor.matmul(pp[:], lhs[k][:],
                                             wx_sb[k][:, s * 512:(s + 1) * 512],
                                             start=(k == 0), stop=False)
                        nc.tensor.matmul(pp[:], ones[:],
                                         bias[:, s * 512:(s + 1) * 512],
                                         start=False, stop=True)
                        if s % 2 == 0:
                            nc.scalar.activation(gout[:, s * 512:(s + 1) * 512],
                                                 pp[:], AF.Copy)
                        else:
                            nc.vector.tensor_copy(gout[:, s * 512:(s + 1) * 512], pp[:])
                    nc.sync.dma_start(gx_d[mt * 128:(mt + 1) * 128, :], gout[:])

                LEAD = 6
                if SCAN_P32:
                    for mt in range(min(LEAD, nmt)):
                        bulk_mtile(mt)

                    def body_cb(body):
                        for mt in (2 * body + LEAD, 2 * body + LEAD + 1):
                            if mt < nmt:
                                bulk_mtile(mt)
                    _scan_layer_p32(nc, tc, l0=False, T=T, gx_dram=gx_d,
                                    whh_sb=whh_sb, zeros=zeros, ident8=ident8,
                                    ifout_dram=ifo_d, body_cb=body_cb)
                else:
                    for mt in range(nmt):
                        bulk_mtile(mt)
                    _scan_layer(nc, tc, l0=False, T=T, gx_dram=gx_d, whh_sb=whh_sb,
                                zeros=zeros, ident8=ident8, ifout_dram=ifo_d)
    nc.compile()
    return nc


def _build_conv(T):
    """conv head + argmax from the layer-2 interface (d1 tiles)."""
    nc = bacc.Bacc("TRN2", target_bir_lowering=False, debug=False,
                   num_devices=NCORES)
    hs_d = [nc.dram_tensor(f"hseq{j}", [128, T * BL], SCAN_DT,
                           kind="ExternalInput").ap() for j in range(2)]
    cw_d = nc.dram_tensor("convwt", [2, 128, NCLS], SCAN_DT, kind="ExternalInput").ap()
    cb_d = nc.dram_tensor("convb", [NCLS, 1], F32, kind="ExternalInput").ap()
    io_d = nc.dram_tensor("iota", [128, NCLS], F32, kind="ExternalInput").ap()
    e81_d = nc.dram_tensor("ident81", [NCLS, NCLS], F32, kind="ExternalInput").ap()
    out_d = nc.dram_tensor("idx", [128, T * BL // 128], I32, kind="ExternalOutput").ap()

    with tile.TileContext(nc) as tc:
        import contextlib
        with contextlib.ExitStack() as ctx:
            wp = ctx.enter_context(tc.tile_pool(name="w", bufs=1))
            rp = ctx.enter_context(tc.tile_pool(name="r", bufs=2))
            pp_ = ctx.enter_context(tc.tile_pool(name="pp", bufs=2, space="PSUM"))
            tpp = ctx.enter_context(tc.tile_pool(name="tq", bufs=4, space="PSUM"))
            op = ctx.enter_context(tc.tile_pool(name="o", bufs=1))
            cw = [wp.tile([128, NCLS], SCAN_DT, tag=f"cw{k}", name=f"cw{k}") for k in range(2)]
            for k in range(2):
                nc.sync.dma_start(cw[k][:], cw_d[k])
            cbias = wp.tile([NCLS, 1], F32)
            nc.sync.dma_start(cbias[:], cb_d[:])
            iota = wp.tile([128, NCLS], F32)
            nc.sync.dma_start(iota[:], io_d[:])
            e81 = wp.tile([NCLS, NCLS], F32)
            nc.sync.dma_start(e81[:], e81_d[:])
            outsb = op.tile([128, T * BL // 128], F32)
            outi = op.tile([128, T * BL // 128], I32)

            nnt = T * BL // 512
            for nt in range(nnt):
                hseq = [rp.tile([128, 512], SCAN_DT, tag=f"hs{k}", name=f"hs{k}") for k in range(2)]
                for k in range(2):
                    nc.sync.dma_start(hseq[k][:], hs_d[k][:, nt * 512:(nt + 1) * 512])
                lp = pp_.tile([NCLS, 512], F32, tag="lg")
                for k in range(2):
                    nc.tensor.matmul(lp[:], cw[k][:], hseq[k][:],
                                     start=(k == 0), stop=(k == 1))
                lg = rp.tile([NCLS, 512], F32, tag="lgs")
                nc.scalar.activation(lg[:], lp[:], AF.Identity, bias=cbias[:])
                for q in range(4):
                    ch = nt * 4 + q
                    tp = tpp.tile([128, NCLS], F32, tag="tr")
                    nc.tensor.transpose(tp[:], lg[0:NCLS, q * 128:(q + 1) * 128], e81[:])
                    lt = rp.tile([128, NCLS], F32, tag="lt")
                    nc.vector.tensor_copy(lt[:], tp[:])
                    mx = rp.tile([128, 1], F32, tag="mx")
                    nc.vector.reduce_max(mx[:], lt[:], axis=mybir.AxisListType.X)
                    msk = rp.tile([128, NCLS], F32, tag="msk")
                    nc.vector.tensor_scalar(msk[:], lt[:], mx[:], None,
                                            mybir.AluOpType.is_equal)
                    nc.vector.tensor_mul(msk[:], msk[:], iota[:])
                    nc.vector.reduce_max(outsb[:, ch:ch + 1], msk[:],
                                         axis=mybir.AxisListType.X)
            nc.vector.tensor_copy(outi[:], outsb[:])
            # raw layout [p=tt*8+b, ch]; host unscrambles to [b, t=ch*16+tt]
            nc.sync.dma_start(out_d, outi[:])
    nc.compile()
    return nc


def _get_progs(T):
    if T not in _prog_cache:
        _prog_cache[T] = (_build_l0(T), _build_mid(T), _build_conv(T))
    return _prog_cache[T]


def _prep_weights(Wih0, Whh0, b0, Wih12, Whh12, b12, conv_w, conv_b):
    """Host-side packing. All gate rows permuted to [i f o g]."""
    f = np.float32
    wx0 = np.zeros((D_IN + 1, 2 * G), f)
    whh0 = np.zeros((2, 128, 2 * G), f)
    for d in range(2):
        wx0[0:D_IN, d * G:(d + 1) * G] = Wih0[d][PERM].T
        wx0[D_IN, d * G:(d + 1) * G] = b0[d][PERM]
        wt = Whh0[d][PERM].T  # [256, 1024]
        for k in range(2):
            whh0[k, :, d * G:(d + 1) * G] = wt[k * 128:(k + 1) * 128]
    wx12 = np.zeros((2, 4, 128, 2 * G), f)
    bias12 = np.zeros((2, 1, 2 * G), f)
    whh12 = np.zeros((2, 2, 128, 2 * G), f)
    for li in range(2):
        for d in range(2):
            wt = Wih12[li, d][PERM].T  # [512, 1024]
            for k in range(4):
                wx12[li, k, :, d * G:(d + 1) * G] = wt[k * 128:(k + 1) * 128]
            bias12[li, 0, d * G:(d + 1) * G] = b12[li, d][PERM]
            ht = Whh12[li, d][PERM].T  # [256, 1024]
            for k in range(2):
                whh12[li, k, :, d * G:(d + 1) * G] = ht[k * 128:(k + 1) * 128]
    cwt = np.zeros((2, 128, NCLS), f)
    for k in range(2):
        cwt[k] = conv_w.T[k * 128:(k + 1) * 128]
    return wx0, whh0, wx12, bias12, whh12, cwt


def _launch(nc, in_maps, cores, trace):
    r = run_bass_kernel_spmd(nc, in_maps, cores, trace=trace)
    ns = r.exec_time_ns if trace else None
    if trace:
        import sys
        tp = r.instructions_and_trace[1] if r.instructions_and_trace else None
        print(f"[launch] exec={ns} ns trace={tp}", file=sys.stderr)
    return r.results, (ns or 0)


def _run(x, Wih0, Whh0, b0, Wih12, Whh12, b12, conv_w, conv_b, trace=False):
    x = np.asarray(x, np.float32)
    args = [np.asarray(a, np.float32) for a in
            (Wih0, Whh0, b0, Wih12, Whh12, b12, conv_w, conv_b)]
    wx0, whh0, wx12, bias12, whh12, cwt = _prep_weights(*args)
    conv_b = args[7]

    T_ = x.shape[2]
    progs = _get_progs(T_)
    nc_l0, nc_mid, nc_conv = progs
    cores = list(range(NCORES))

    zeros = np.zeros((128, 32), SCAN_NP)
    ident8 = np.eye(64, dtype=SCAN_NP)
    ones = np.ones((1, 128), SCAN_NP)
    iota = np.tile(np.arange(NCLS, dtype=np.float32), (128, 1))
    ident81 = np.eye(NCLS, dtype=np.float32)

    # xt per core: [65, T*8], col = t*8 + b, row 64 = ones
    xt_all = []
    for ci in range(NCORES):
        xs = x[ci * BL:(ci + 1) * BL]          # [8, 64, T]
        xt = np.zeros((D_IN + 1, T_ * BL + 32), SCAN_NP)
        xt[0:D_IN, 0:T_ * BL] = xs.transpose(1, 2, 0).reshape(D_IN, T_ * BL)
        xt[D_IN, 0:T_ * BL] = 1.0
        xt_all.append(xt)

    total_ns = 0
    r0, ns = _launch(nc_l0, [
        {"xt": xt_all[ci], "wx0": wx0.astype(SCAN_NP), "whh": whh0.astype(SCAN_NP), "zeros": zeros,
         "ident8": ident8} for ci in cores], cores, trace)
    total_ns += ns

    cur = r0
    for li in range(2):
        rm, ns = _launch(nc_mid, [
            {**{f"ifin{j}": cur[ci][f"ifout{j}"] for j in range(4)},
             "wx": wx12[li].astype(SCAN_NP), "bias": bias12[li].astype(SCAN_NP), "ones": ones,
             "whh": whh12[li].astype(SCAN_NP), "zeros": zeros, "ident8": ident8}
            for ci in cores], cores, trace)
        cur = rm
        total_ns += ns

    rc, ns = _launch(nc_conv, [
        {"hseq0": cur[ci]["ifout2"], "hseq1": cur[ci]["ifout3"],
         "convwt": cwt.astype(SCAN_NP), "convb": conv_b.reshape(NCLS, 1).astype(np.float32),
         "iota": iota, "ident81": ident81} for ci in cores], cores, trace)
    total_ns += ns

    outs = []
    for ci in cores:
        raw = rc[ci]["idx"]                      # [128, T/16]: [tt*8+b, ch]
        o = raw.reshape(16, BL, T_ // 16).transpose(1, 2, 0).reshape(BL, T_)
        outs.append(o)
    return np.concatenate(outs, axis=0).astype(np.int32), total_ns


def kernel(**inputs):
    out, _ = _run(**inputs)
    return out


def profiled_run(**inputs):
    _, ns = _run(**inputs, trace=True)
    return ns

